# revision 1
# baseline (speedup 1.0000x reference)
"""Trainium2 Bass kernel for: ConvTranspose3d(16->64, k=4, s=2, p=1) + conv_bias,
mean over depth, + bias, channel softmax, tanh, *2.

Input  x: (16, 16, 16, 32, 32) f32  -> Output: (16, 64, 1, 64, 64) f32.

Algorithm (per batch):
  The depth mean commutes with the transposed conv:
    mean_d' ConvT3D(x, w) = ConvT2D(A, W2) / 32
  with A = [sum_d x, x[:,0], x[:,15]] (48 channels) and
  W2 = [sum_kd w, -w[kd=0], -w[kd=3]] / 32, because the only (d, kd) pairs
  whose output depth 2d-1+kd falls outside [0, 32) are (0,0) and (15,3).

  The stride-2 ConvT2D splits into 4 output-parity phases (ph, pw), each a
  2x2-tap conv over a padded A. The two h-taps are stacked on SBUF
  partitions (B = [A_pad @ 0:48; A_pad shifted down one row @ 64:112]) so
  one matmul contracts K=112 (rows 48:64 zero); the two w-taps accumulate
  in PSUM. Each phase's [64, 32, 32] result is exp'd (+bias) into the right
  partition-half / w-parity slice of a [128=(ph,ch), 32, 64] SBUF tile, so
  the softmax epilogue (channel-sum matmul -> reciprocal -> broadcast
  matmul -> mul -> tanh -> *2) runs at full 128-partition width and the
  output DMA is row-contiguous.

  Matmuls run as float32r (~tf32 PE mode, 4x fp32 throughput); all fp32r
  operands are produced by casting DMAs or compute ops as the BIR verifier
  requires. Measured end-to-end rel err vs the fp32 reference: 2.7e-4.

Sharding: data-parallel over batch, 2 batches per core on 8 cores.
"""

import numpy as np

import concourse.bacc as bacc
import concourse.mybir as mybir
import concourse.tile as tile
from concourse.bass_utils import run_bass_kernel_spmd

# Problem constants (hardcoded; kernel.py must be self-contained).
B_TOTAL = 16
IN_C, OUT_C = 16, 64
D_IN, H_IN, W_IN = 16, 32, 32
KK, STRIDE, PAD = 4, 2, 1
SCALE = 2.0
D_OUT = 32  # conv output depth (before mean)
N_CORES = 8
B_LOC = B_TOTAL // N_CORES  # batches per core

F32 = mybir.dt.float32
F32R = mybir.dt.float32r

USE_F32R = True  # fast fp32 matmul mode (1 cyc/row vs 4); flip if accuracy fails


def _pe_dt():
    """dtype for TensorEngine-facing tiles; fp32r producers must be typed."""
    return F32R if USE_F32R else F32


def _pe_view(ap):
    """view a PE-facing (possibly f32r) tile as plain f32 for DVE/ACT reads"""
    return ap.bitcast(F32) if USE_F32R else ap


def build_bass(repeat=1):
    """repeat>1 re-runs the whole per-core workload in one NEFF (for timing:
    wall(L) - wall(1) isolates device time from dispatch overhead)."""
    nc = bacc.Bacc(name="deconv_mean_softmax")

    x_d = nc.dram_tensor("x", [B_LOC, IN_C, D_IN, H_IN, W_IN], F32, kind="ExternalInput")
    wsel_d = nc.dram_tensor("wsel", [128, 96], F32, kind="ExternalInput")
    wk_d = nc.dram_tensor("wk", [112, 8, 64], F32, kind="ExternalInput")
    bias_d = nc.dram_tensor("bias2", [128, 1], F32, kind="ExternalInput")
    sel2_d = nc.dram_tensor("sel2", [2, 128], F32, kind="ExternalInput")
    oneh_d = nc.dram_tensor("onehot2", [128, 2], F32, kind="ExternalInput")
    bz_d = nc.dram_tensor("bzero", [128, 34 * 34], F32, kind="ExternalInput")
    out_d = nc.dram_tensor("out", [B_LOC, OUT_C, 64, 64], F32, kind="ExternalOutput")

    with tile.TileContext(nc) as tc:
        with (
            tc.tile_pool(name="consts", bufs=1) as consts,
            tc.tile_pool(name="xin", bufs=4) as xin,
            tc.tile_pool(name="epool", bufs=3) as epool,
            tc.tile_pool(name="opool", bufs=3) as opool,
            tc.tile_pool(name="spool", bufs=6) as spool,
            # separate PSUM pools so batch b+1's prep matmuls don't wait on
            # batch b's softmax tiles: conv 2x2 banks + prep 2x1 + softmax 2x1
            tc.tile_pool(name="psum_conv", bufs=2, space="PSUM") as psum_conv,
            tc.tile_pool(name="psum_a", bufs=2, space="PSUM") as psum_a,
            tc.tile_pool(name="psum_sr", bufs=2, space="PSUM") as psum_sr,
        ):
            PED = _pe_dt()
            # f32r-typed tiles must be produced by casting (gpsimd) DMAs or
            # compute ops so the BIR verifier sees rounded fp32r inputs
            pe_dma = nc.gpsimd.dma_start if USE_F32R else nc.sync.dma_start

            # Constants
            wsel = consts.tile([128, 96], PED)
            pe_dma(out=wsel, in_=wsel_d[:, :])
            wk = consts.tile([112, 8, 64], PED)
            pe_dma(out=wk, in_=wk_d[:, :, :])
            bias2 = consts.tile([128, 1], F32)
            nc.sync.dma_start(out=bias2, in_=bias_d[:, :])
            # one-hot block lhsTs for channel sum / broadcast
            onehot2 = consts.tile([128, 2], PED)
            pe_dma(out=onehot2, in_=oneh_d[:, :])
            sel2 = consts.tile([2, 128], PED)
            pe_dma(out=sel2, in_=sel2_d[:, :])

            # Two persistent B slots, zeroed once from host; the batch loop
            # only rewrites the interiors, so padding and the zero partition
            # rows 48:64 stay zero for the whole kernel.
            B_slots = []
            for i in range(3):
                bs = consts.tile([128, 34, 34], PED, tag=f"Bslot{i}")
                pe_dma(out=bs.rearrange("p a b -> p (a b)"), in_=bz_d[:, :])
                B_slots.append(bs)

            # kh pairs per ph (block0 tap, block1 tap); kw pairs per pw
            KH = {0: (1, 3), 1: (0, 2)}
            KW = {0: (1, 3), 1: (0, 2)}

            for rep in range(repeat):
              otiles = []
              # ---- prep stage for BOTH batches first (software pipeline):
              # x loads, rounding, selector matmuls, B assembly — hoisted so
              # they overlap the other batch's conv/softmax ----
              bts = []
              for b in range(B_LOC):
                # ---- load x: two tiles, partitions = (d-in-tile, c) ----
                xt = []
                for t in range(2):
                    src = (
                        x_d[b]
                        .rearrange("c d h w -> d c (h w)")[t * 8 : (t + 1) * 8]
                    )
                    if USE_F32R:
                        # HWDGE load (fast) + compute-op rounding pass to f32r
                        # (casting DMA would ride the slow software DGE)
                        xraw = xin.tile([128, 1024], F32, tag="xraw")
                        nc.sync.dma_start(out=xraw, in_=src)
                        xtile = xin.tile([128, 1024], PED, tag="xt")
                        nc.vector.tensor_copy(out=xtile, in_=xraw)
                    else:
                        xtile = xin.tile([128, 1024], PED, tag="xt")
                        nc.sync.dma_start(out=xtile, in_=src)
                    xt.append(xtile)

                # ---- A = [sum_d x, x[:,0], x[:,15]] via selector matmul ----
                # B stack tile: block0 @ parts 0:48, block1 @ parts 64:112
                # (engine APs must start at 32-aligned partitions; rows 48:64
                # stay zero and get zero weights)
                Bt = B_slots[(rep * B_LOC + b) % 3]
                for q in range(2):  # h-row halves of A
                    psA = psum_a.tile([48, 512], F32, tag="ps_a")
                    for t in range(2):
                        nc.tensor.matmul(
                            psA,
                            wsel[:, t * 48 : (t + 1) * 48],
                            xt[t][:, q * 512 : (q + 1) * 512],
                            start=(t == 0),
                            stop=(t == 1),
                        )
                    psA3 = psA.rearrange("p (h w) -> p h w", w=32)
                    # copy into both partition blocks of B (block1 shifted +1
                    # row), both on DVE: ACT (exp/tanh) is the critical engine
                    nc.vector.tensor_copy(
                        out=Bt[0:48, 1 + 16 * q : 17 + 16 * q, 1:33], in_=psA3
                    )
                    nc.vector.tensor_copy(
                        out=Bt[64:112, 2 + 16 * q : 18 + 16 * q, 1:33], in_=psA3
                    )
                bts.append(Bt)

              for b in range(B_LOC):
                Bt = bts[b]
                # ---- conv phases; results land in E = exp(conv+bias).
                # Softmax runs per w'-parity right after that parity's two
                # exps, overlapping the other parity's conv matmuls. ----
                Et = epool.tile([128, 32, 64], PED, tag="E")
                Et4 = Et.rearrange("p m (w two) -> p m w two", two=2)
                Ot = opool.tile([128, 32, 64], F32, tag="O")
                Ot4 = Ot.rearrange("p m (w two) -> p m w two", two=2)
                for pw in (0, 1):
                    for ph in (0, 1):
                        # fp32r matmul requires dst base partition 0, so each
                        # (ph, pw) phase gets its own [64, 32, 32] PSUM tile
                        psC = psum_conv.tile([64, 32, 32], F32, tag="conv")
                        row_off = 1 + ph
                        for q in range(2):  # 16 m-rows => one bank each
                            outsl = psC[:, 16 * q : 16 * q + 16, :]
                            for tap in (0, 1):
                                col_off = (1 + pw) - tap
                                widx = (ph * 2 + pw) * 2 + tap
                                rhs = Bt[
                                    0:112,
                                    row_off + 16 * q : row_off + 16 * q + 16,
                                    col_off : col_off + 32,
                                ]
                                nc.tensor.matmul(
                                    outsl,
                                    wk[:, widx, :],
                                    rhs,
                                    start=(tap == 0),
                                    stop=(tap == 1),
                                )
                        # E[ph-half, :, pw::2] = exp(conv + bias)
                        nc.scalar.activation(
                            out=Et4[ph * 64 : ph * 64 + 64, :, :, pw], in_=psC,
                            func=mybir.ActivationFunctionType.Exp,
                            bias=bias2[ph * 64 : ph * 64 + 64], scale=1.0,
                        )
                    # softmax over this parity's columns (both ph halves ready)
                    for j in range(2):  # 16 m-rows x 32 w-parity cols = 512
                        Ej = Et4[:, 16 * j : 16 * j + 16, :, pw]
                        psS = psum_sr.tile([2, 512], F32, tag="ps_sr")
                        nc.tensor.matmul(psS, onehot2, Ej)
                        # ~18-bit reciprocal is ~5x faster than exact on DVE
                        # and far above the f32r noise floor; denominators are
                        # sums of 64 positive exps, safely inside its domain
                        Rt = spool.tile([2, 512], PED, tag="R")
                        if USE_F32R:
                            Rf = spool.tile([2, 512], F32, tag="Rf")
                            nc.vector.reciprocal_approx_fast(out=Rf, in_=psS)
                            nc.vector.tensor_copy(out=Rt, in_=Rf)
                        else:
                            nc.vector.reciprocal(out=Rt, in_=psS)
                        # broadcast R back to 128 partitions
                        psR = psum_sr.tile([128, 512], F32, tag="ps_sr")
                        nc.tensor.matmul(psR, sel2, Rt)
                        # softmax = E * bcast(1/S)
                        Dj = Ot4[:, 16 * j : 16 * j + 16, :, pw]
                        nc.vector.tensor_mul(Dj, _pe_view(Ej), psR)
                otiles.append((b, Ot))

              # tanh for both batches grouped at the end: Exp and Tanh live in
              # different ACT table sets (1283ns reload each), so alternating
              # exp/tanh per batch costs 2 extra reloads per workload
              for b, Ot in otiles:
                Of = Ot.rearrange("p h w -> p (h w)")
                nc.scalar.activation(
                    out=Of, in_=Of, func=mybir.ActivationFunctionType.Tanh,
                )
                nc.vector.tensor_scalar_mul(Of, Of, SCALE)

                # ---- store: partitions 0:64 are even h', 64:128 odd h' ----
                o3 = out_d[b].rearrange("c (h ph) w -> c h ph w", ph=2)
                nc.sync.dma_start(out=o3[:, :, 0, :], in_=Ot[0:64])
                nc.sync.dma_start(out=o3[:, :, 1, :], in_=Ot[64:128])

    return nc


def host_constants(weight, conv_bias, bias):
    w = np.asarray(weight, np.float32).astype(np.float64)
    W2 = np.empty((48, OUT_C, KK, KK), np.float64)
    W2[0:16] = w.sum(axis=2) / D_OUT
    W2[16:32] = -w[:, :, 0] / D_OUT
    W2[32:48] = -w[:, :, 3] / D_OUT

    KH = {0: (1, 3), 1: (0, 2)}
    KW = {0: (1, 3), 1: (0, 2)}
    wk = np.zeros((112, 8, 64), np.float64)
    for ph in (0, 1):
        for pw in (0, 1):
            for tap in (0, 1):
                widx = (ph * 2 + pw) * 2 + tap
                wk[0:48, widx, :] = W2[:, :, KH[ph][0], KW[pw][tap]]
                wk[64:112, widx, :] = W2[:, :, KH[ph][1], KW[pw][tap]]

    # selector for A = [sum_d x, x[:,0], x[:,15]]: two [128, 48] blocks
    wsel = np.zeros((128, 96), np.float64)
    for t in range(2):
        for dd in range(8):
            d = t * 8 + dd
            for c in range(IN_C):
                p = dd * IN_C + c
                wsel[p, t * 48 + c] = 1.0  # sum_d
                if d == 0:
                    wsel[p, t * 48 + 16 + c] = 1.0  # x[:, 0]
                if d == 15:
                    wsel[p, t * 48 + 32 + c] = 1.0  # x[:, 15]

    bias_comb = (
        np.asarray(conv_bias, np.float64) + np.asarray(bias, np.float64).reshape(-1)
    )
    bias2 = np.tile(bias_comb, 2).reshape(128, 1)
    sel2 = np.zeros((2, 128), np.float32)
    sel2[0, 0:64] = 1.0
    sel2[1, 64:128] = 1.0
    onehot2 = np.zeros((128, 2), np.float32)
    onehot2[0:64, 0] = 1.0
    onehot2[64:128, 1] = 1.0
    bzero = np.zeros((128, 34 * 34), np.float32)
    return {
        "wsel": wsel.astype(np.float32),
        "wk": wk.astype(np.float32),
        "bias2": bias2.astype(np.float32),
        "sel2": sel2,
        "onehot2": onehot2,
        "bzero": bzero,
    }


_CACHED = {}


def kernel(x, weight, conv_bias, bias):
    x = np.ascontiguousarray(np.asarray(x, np.float32))
    consts = host_constants(weight, conv_bias, bias)

    if "nc" not in _CACHED:
        nc = build_bass()
        nc.finalize()
        _CACHED["nc"] = nc
    nc = _CACHED["nc"]

    in_maps = []
    for core in range(N_CORES):
        xs = np.ascontiguousarray(x[core * B_LOC : (core + 1) * B_LOC])
        in_maps.append({"x": xs, **consts})

    res = run_bass_kernel_spmd(nc, in_maps, core_ids=list(range(N_CORES)))
    outs = [r["out"] for r in res.results]
    full = np.concatenate(outs, axis=0)  # (16, 64, 64, 64)
    return full[:, :, None, :, :]




if __name__ == "__main__":
    import reference

    inputs = reference.setup_inputs()
    out = kernel(**{k: np.asarray(v) for k, v in inputs.items()})
    print("kernel out", out.shape, out.dtype)



# revision 23
# speedup vs baseline: 70.8284x; 70.8284x over previous
"""Trainium2 Bass kernel for: ConvTranspose3d(16->64, k=4, s=2, p=1) + conv_bias,
mean over depth, + bias, channel softmax, tanh, *2.

Input  x: (16, 16, 16, 32, 32) f32  -> Output: (16, 64, 1, 64, 64) f32.

v5 design (bf16, block-diagonal phase pairing, raw-layout stores):
  Depth mean commutes with the transposed conv:
    mean_d' ConvT3D(x, w) = ConvT2D(A, W2) / 32
  with A = [sum_d x, x[:,0], x[:,15]] (48 channels) and
  W2 = [sum_kd w, -w[kd=0], -w[kd=3]] / 32.

  The stride-2 ConvT2D splits into 4 output-parity phases (ph, pw). B
  stacks A twice on SBUF partitions: block0 @ 0:48 holds A shifted down
  one row (B0[r] = A[r-1], from PSUM), block1 @ 64:112 holds A unshifted
  (cheap bf16 SBUF copy of block0). A block-diagonal lhsT [112, 128]
  computes BOTH h-parities row-aligned in one pass: cols 0:64 (ph=0) take
  kh-tap 1 (resp. 3) from block0, cols 64:128 (ph=1) take kh-tap 0
  (resp. 2) from block1, via two rhs streams at row offsets 1+m / m.
  With the two kw-taps that is 4 accumulating matmuls per (w-parity,
  16-row chunk) into a full [128=(ph,ch), 16, 32] PSUM slice.

  Per w-parity pw (pipelined against the other parity's conv): one
  [128, 1024] exp(conv+bias) -> E[:, pw] (bf16), channel sums via a
  one-hot matmul -> psS [2, 1024], fast reciprocal (DVE, f32), then a
  single SWDGE casting broadcast-DMA replicates the two f32 R rows into
  a bf16 [128, 1024] tile (descriptor work on the idle GpSimd queue; no
  broadcast matmul, no PSUM-read multiply, no separate squeeze op). One
  bf16 2x multiply, tanh into the output tile, and an in-place *2.

  Output leaves the device in raw (ph,ch)/(pw,m,w) layout as ONE
  contiguous bf16 DMA per batch; the host de-interleaves the parities
  and casts to f32 while gathering shards. All PE-facing constants are
  pre-cast to bf16 on the host and arrive as one packed blob; B padding
  is zeroed by DVE/ACT memsets, so nothing rides the slow paths.

Sharding: data-parallel over batch, 2 batches per core on 8 cores.
"""

import numpy as np
from ml_dtypes import bfloat16

import concourse.bacc as bacc
import concourse.mybir as mybir
import concourse.tile as tile
from concourse.bass_utils import run_bass_kernel_spmd

# Problem constants (hardcoded; kernel.py must be self-contained).
B_TOTAL = 16
IN_C, OUT_C = 16, 64
D_IN, H_IN, W_IN = 16, 32, 32
KK, STRIDE, PAD = 4, 2, 1
SCALE = 2.0
D_OUT = 32  # conv output depth (before mean)
N_CORES = 8
B_LOC = B_TOTAL // N_CORES  # batches per core

F32 = mybir.dt.float32
BF16 = mybir.dt.bfloat16

# kw pairs per pw (tap order: col_off = 2 + pw - tap with 2-col left pad)
KW = {0: (1, 3), 1: (0, 2)}

N_BSLOTS = 2
BLOB_W = 96 + 8 * 128 + 2  # wsel | wk | onehot2, packed bf16


def build_bass(repeat=1, hw_loop=False):
    """repeat>1 re-runs the whole per-core workload (unrolled, or as a
    hardware For_i loop when hw_loop=True) for wall-clock differencing."""
    nc = bacc.Bacc(name="deconv_mean_softmax")

    x_d = nc.dram_tensor("x", [B_LOC, IN_C, D_IN, H_IN, W_IN], BF16, kind="ExternalInput")
    blob_d = nc.dram_tensor("blob", [128, BLOB_W], BF16, kind="ExternalInput")
    bias_d = nc.dram_tensor("bias2", [128, 1], F32, kind="ExternalInput")
    # raw layout: [b, (ph,ch), pw, m, w]; host de-interleaves
    out_d = nc.dram_tensor("out", [B_LOC, 128, 2, 32, 32], BF16, kind="ExternalOutput")

    with tile.TileContext(nc) as tc:
        with (
            tc.tile_pool(name="consts", bufs=1) as consts,
            tc.tile_pool(name="xin", bufs=3) as xin,
            tc.tile_pool(name="epool", bufs=2) as epool,
            tc.tile_pool(name="opool", bufs=2) as opool,
            tc.tile_pool(name="spool", bufs=4) as spool,
            tc.tile_pool(name="psum_conv", bufs=2, space="PSUM") as psum_conv,
            tc.tile_pool(name="psum_a", bufs=1, space="PSUM") as psum_a,
            tc.tile_pool(name="psum_s", bufs=1, space="PSUM") as psum_s,
        ):
            blob = consts.tile([128, BLOB_W], BF16)
            nc.sync.dma_start(out=blob, in_=blob_d[:, :])
            bias2 = consts.tile([128, 1], F32)
            nc.sync.dma_start(out=bias2, in_=bias_d[:, :])
            wsel = blob[:, 0:96]
            wk = blob[0:112, 96 : 96 + 1024].rearrange("p (a b) -> p a b", b=128)
            onehot2 = blob[:, 96 + 1024 : 96 + 1026]

            # Persistent B slots [128, 34 rows, 36 cols], interior cols 2:34
            # (4B-aligned for the bf16 block1 copy). Zeroed once by memsets;
            # the batch loop only rewrites interiors, so padding and
            # partition rows 48:64 stay zero for the whole kernel.
            B_slots = []
            for i in range(N_BSLOTS):
                bs = consts.tile([128, 34, 36], BF16, tag=f"Bslot{i}")
                flat = bs.rearrange("p a b -> p (a b)")
                if i == 0:
                    nc.vector.memset(flat, 0.0)
                else:
                    nc.scalar.memzero(flat)
                B_slots.append(bs)

            def body(rep):
                bts = []
                # ---- prep stage for BOTH batches first (software pipeline) ----
                for b in range(B_LOC):
                    xt = []
                    for t in range(2):
                        src = (
                            x_d[b]
                            .rearrange("c d h w -> d c (h w)")[t * 8 : (t + 1) * 8]
                        )
                        xtile = xin.tile([128, 1024], BF16, tag="xt")
                        nc.sync.dma_start(out=xtile, in_=src)
                        xt.append(xtile)

                    # A = [sum_d x, x[:,0], x[:,15]] via selector matmul,
                    # both q-halves into one 2-bank psA [48, 1024]
                    Bt = B_slots[(rep * B_LOC + b) % N_BSLOTS]
                    psA = psum_a.tile([48, 1024], F32, tag="ps_a")
                    for q in range(2):
                        for t in range(2):
                            nc.tensor.matmul(
                                psA[:, q * 512 : (q + 1) * 512],
                                wsel[:, t * 48 : (t + 1) * 48],
                                xt[t][:, q * 512 : (q + 1) * 512],
                                start=(t == 0),
                                stop=(t == 1),
                            )
                    psA3 = psA.rearrange("p (h w) -> p h w", w=32)
                    # block0 = A shifted down one row (PSUM->SBUF, on ACT),
                    # block1 = A unshifted via cheap bf16 SBUF copy (DVE)
                    nc.scalar.copy(out=Bt[0:48, 1:33, 2:34], in_=psA3)
                    nc.vector.tensor_copy(
                        out=Bt[64:112, 0:32, 2:34], in_=Bt[0:48, 1:33, 2:34]
                    )
                    bts.append(Bt)

                for b in range(B_LOC):
                    Bt = bts[b]
                    # E/Of layout: [128=(ph,ch), 2=pw, 32=m, 32=w]
                    Et = epool.tile([128, 2, 32, 32], BF16, tag="E")
                    Of = opool.tile([128, 2, 32, 32], BF16, tag="Of")
                    for pw in (0, 1):
                        psC = psum_conv.tile([128, 32, 32], F32, tag="conv")
                        for q in range(2):
                            outsl = psC[:, 16 * q : 16 * q + 16, :]
                            for tap in (0, 1):
                                co = (2 + pw) - tap
                                for ab in (0, 1):  # rhs row offset 1+m / m
                                    rhs = Bt[
                                        0:112,
                                        (1 - ab) + 16 * q : (1 - ab) + 16 * q + 16,
                                        co : co + 32,
                                    ]
                                    nc.tensor.matmul(
                                        outsl,
                                        wk[:, (pw * 2 + tap) * 2 + ab, :],
                                        rhs,
                                        start=(tap == 0 and ab == 0),
                                        stop=(tap == 1 and ab == 1),
                                    )
                        # Epilogue; the very last phase runs 16-row chunked so
                        # the kernel tail drains as a short pipeline instead
                        # of one long serial chain.
                        last = b == B_LOC - 1 and pw == 1
                        CHUNKS = ((0, 16), (16, 32)) if last else ((0, 32),)
                        psS = psum_s.tile([2, 32, 32], F32, tag="ps_s")
                        for m0, m1 in CHUNKS:
                            n = (m1 - m0) * 32
                            # E[:, pw] = exp(conv + bias)
                            nc.scalar.activation(
                                out=Et[:, pw, m0:m1], in_=psC[:, m0:m1],
                                func=mybir.ActivationFunctionType.Exp,
                                bias=bias2, scale=1.0,
                            )
                            # channel sums for both ph halves
                            for q0 in range(m0, m1, 16):
                                nc.tensor.matmul(
                                    psS[:, q0 : q0 + 16, :],
                                    onehot2,
                                    Et[:, pw, q0 : q0 + 16, :],
                                )
                            # ~18-bit reciprocal (sums of 64 positive exps
                            # are safely inside its domain)
                            Rf = spool.tile([2, 1024], F32, tag="Rf")
                            nc.vector.reciprocal_approx_fast(
                                out=Rf[:, : n],
                                in_=psS[:, m0:m1].rearrange("p a b -> p (a b)"),
                            )
                            # casting broadcast-DMA replicates the two R rows
                            # across the 64-partition halves (SWDGE queue)
                            Rb = spool.tile([128, 1024], BF16, tag="Rb")
                            nc.gpsimd.dma_start(
                                out=Rb[:, : n],
                                in_=Rf[:, : n].unsqueeze(1).broadcast_to([2, 64, n]),
                            )
                            # softmax = E * bcast(1/S); tanh + *2 into output
                            Ot = Of[:, pw, m0:m1].rearrange("p a b -> p (a b)")
                            nc.vector.tensor_mul(
                                Ot,
                                Et[:, pw, m0:m1].rearrange("p a b -> p (a b)"),
                                Rb[:, : n],
                            )
                            nc.scalar.activation(
                                out=Ot, in_=Ot,
                                func=mybir.ActivationFunctionType.Tanh,
                            )
                            nc.vector.tensor_scalar_mul(Ot, Ot, SCALE)
                            # raw-layout store as soon as the chunk is ready
                            nc.sync.dma_start(
                                out=out_d[b, :, pw, m0:m1], in_=Of[:, pw, m0:m1]
                            )

            if hw_loop and repeat > 1:
                with tc.For_i(0, repeat, 1):
                    body(0)
            else:
                for rep in range(repeat):
                    body(rep)

    return nc


def host_constants(weight, conv_bias, bias):
    w = np.asarray(weight, np.float32).astype(np.float64)
    W2 = np.empty((48, OUT_C, KK, KK), np.float64)
    W2[0:16] = w.sum(axis=2) / D_OUT
    W2[16:32] = -w[:, :, 0] / D_OUT
    W2[32:48] = -w[:, :, 3] / D_OUT

    # block-diagonal paired weights: widx = (pw*2 + tap)*2 + ab
    #   ab=0 (rhs rows 1+m): ph0 <- kh1 on block0, ph1 <- kh0 on block1
    #   ab=1 (rhs rows   m): ph0 <- kh3 on block0, ph1 <- kh2 on block1
    wk = np.zeros((112, 8, 128), np.float64)
    for pw in (0, 1):
        for tap in (0, 1):
            kw = KW[pw][tap]
            wk[0:48, (pw * 2 + tap) * 2 + 0, 0:64] = W2[:, :, 1, kw]
            wk[64:112, (pw * 2 + tap) * 2 + 0, 64:128] = W2[:, :, 0, kw]
            wk[0:48, (pw * 2 + tap) * 2 + 1, 0:64] = W2[:, :, 3, kw]
            wk[64:112, (pw * 2 + tap) * 2 + 1, 64:128] = W2[:, :, 2, kw]

    # selector for A = [sum_d x, x[:,0], x[:,15]]: two [128, 48] blocks
    wsel = np.zeros((128, 96), np.float64)
    for t in range(2):
        for dd in range(8):
            d = t * 8 + dd
            for c in range(IN_C):
                p = dd * IN_C + c
                wsel[p, t * 48 + c] = 1.0  # sum_d
                if d == 0:
                    wsel[p, t * 48 + 16 + c] = 1.0  # x[:, 0]
                if d == 15:
                    wsel[p, t * 48 + 32 + c] = 1.0  # x[:, 15]

    onehot2 = np.zeros((128, 2), np.float64)
    onehot2[0:64, 0] = 1.0
    onehot2[64:128, 1] = 1.0

    blob = np.zeros((128, BLOB_W), np.float64)
    blob[:, 0:96] = wsel
    blob[0:112, 96 : 96 + 1024] = wk.reshape(112, 1024)
    blob[:, 96 + 1024 : 96 + 1026] = onehot2

    bias_comb = (
        np.asarray(conv_bias, np.float64) + np.asarray(bias, np.float64).reshape(-1)
    )
    bias2 = np.tile(bias_comb, 2).reshape(128, 1)
    return {
        "blob": blob.astype(bfloat16),
        "bias2": bias2.astype(np.float32),
    }


_CACHED = {}


def kernel(x, weight, conv_bias, bias):
    x = np.asarray(x, np.float32).astype(bfloat16)
    consts = host_constants(weight, conv_bias, bias)

    if "nc" not in _CACHED:
        nc = build_bass()
        nc.finalize()
        _CACHED["nc"] = nc
    nc = _CACHED["nc"]

    in_maps = []
    for core in range(N_CORES):
        xs = np.ascontiguousarray(x[core * B_LOC : (core + 1) * B_LOC])
        in_maps.append({"x": xs, **consts})

    res = run_bass_kernel_spmd(nc, in_maps, core_ids=list(range(N_CORES)))
    # raw [B_LOC, (ph,ch), pw, m, w] -> [B, ch, (m,ph), (w,pw)]
    raw = np.concatenate([r["out"] for r in res.results], axis=0)
    raw = raw.reshape(B_TOTAL, 2, 64, 2, 32, 32).astype(np.float32)
    full = raw.transpose(0, 2, 4, 1, 5, 3).reshape(B_TOTAL, 64, 64, 64)
    return np.ascontiguousarray(full[:, :, None, :, :])


if __name__ == "__main__":
    import reference

    inputs = reference.setup_inputs()
    out = kernel(**{k: np.asarray(v) for k, v in inputs.items()})
    print("kernel out", out.shape, out.dtype)
